# revision 9
# baseline (speedup 1.0000x reference)
"""ConvergedInhibition TRN2 kernel.

The reference computes, per pixel (n,h,w), an FFT deconvolution along the
channel axis: y = ifft(fft(x)/fft(k)).real. Since k is fixed, this is a
circular convolution with g = ifft(1/fft(k)): y[i] = sum_j g[(i-j) mod C] x[j]
— a dense CxC circulant matmul applied to every pixel. Viewing activations[n]
as a [C, H*W] matrix A_n, the problem is out_n = G @ A_n: a [512,512] x
[512,3136] matmul per image, data-parallel over 32 images across 8 cores.

Implementation choices (measured on HW):
- The deconv kernel g is concentrated in a ~224-wide circular window.
  Rotating output rows by S=288 (z[r] = y[(r+S) mod C]) aligns the support
  so only 2 of 4 K-chunks of the contraction carry mass (each output row
  keeps a 256-wide sliding window of g; truncation costs ~2e-3 rel).
  The rotation is undone by a host-side gather.
- fp8 (e3m4) I/O: |x| < 6 << 15.5 = e3m4 max, 4 mantissa bits -> ~1.34e-2
  rms rounding per side (measured end-to-end rel err 1.907e-2, HW matches
  the numpy simulation exactly). Per-zc output dtype stays configurable.
  Weights are fp16 (PE upcasts operands to FP22, mixed dtypes allowed).
- Only the 8 needed [128,128] weight blocks ship (256 KB, one DMA).
- Each dma_start occupies its HWDGE ring ~630ns regardless of size, so
  DMAs are as large as possible: one per (img, cb) on loads (400KB, 3D
  access pattern covering all 4 channel chunks), half-image-width stores.
  All loads+stores ride the sync ring; gt + the first block are prefetched
  from the semaphore-clears block so their sems fire before the main
  block's barrier lifts.
- PSUM->out casts are split between DVE (zc 0,1) and ACT (zc 2,3), each
  with its own completion semaphore (bank t%8 is always cast by the same
  engine since t and t-8 share (zc, p2)), so cast rate keeps up with PE.
"""

import numpy as np
import ml_dtypes

import concourse.bass as bass  # noqa: F401  (registers bass types)
import concourse.mybir as mybir
from concourse import bacc
from concourse.bass_utils import run_bass_kernel_spmd

N_CORES = 8
N, C, H, W = 32, 512, 56, 56
HW = H * W                      # 3136
IMGS = N // N_CORES             # 4 images per core
P = 128                         # partitions
NCHUNK = C // P                 # 4
PT = 392                        # pixel tile (free dim), 3136 = 8*392
NPT = HW // PT                  # 8
CB = 784                        # column block, 3136 = 4*784
NCB = HW // CB                  # 4
ROT = 288                       # output-row rotation aligning g's support
KEPT_D = (0, 1)                 # kept (zc - jc) mod 4 chunk distances
NKEPT = len(KEPT_D)

F8 = mybir.dt.float8e3
F16 = mybir.dt.float16
NP_OF = {F8: ml_dtypes.float8_e3m4, F16: np.float16}
A_DT = F8                       # activation (input) dtype
OUT_DT = [F8, F8, F8, F8]       # output dtype per output chunk zc
W_DT = F16                      # gt weights
CAST_ENG = ["v", "v", "a", "a"]  # cast engine per zc (v=DVE, a=ACT)

P2 = NPT // NCB                       # p-tiles per column block (2)
TILES_PER_CB = NCHUNK * P2            # 8 psum tiles per (img, cb)
TILES_PER_IMG = NCB * TILES_PER_CB    # 32
NTILES = IMGS * TILES_PER_IMG         # 128


def tidx(img, cb, zc, p2):
    return img * TILES_PER_IMG + cb * TILES_PER_CB + zc * P2 + p2


def zc_of(t):
    return (t % TILES_PER_CB) // P2


# cnt_eng[e][t]: number of tiles t' <= t whose cast runs on engine e
_cnt = {"v": [0] * NTILES, "a": [0] * NTILES}
_c = {"v": 0, "a": 0}
for _t in range(NTILES):
    _c[CAST_ENG[zc_of(_t)]] += 1
    _cnt["v"][_t] = _c["v"]
    _cnt["a"][_t] = _c["a"]

_CACHE = {}


def _build_nc():
    nc = bacc.Bacc("TRN2", target_bir_lowering=False, debug=False,
                   num_devices=N_CORES)
    act = nc.dram_tensor("act", [IMGS, C, HW], A_DT, kind="ExternalInput")
    gtp = nc.dram_tensor("gtp", [P, NCHUNK * NKEPT * P], W_DT,
                         kind="ExternalInput")
    out = [nc.dram_tensor(f"out{zc}", [IMGS, P, HW], OUT_DT[zc],
                          kind="ExternalOutput") for zc in range(NCHUNK)]

    # [img, p, jc, m]: partition-major view of the (jc p) channel split so
    # one DMA per (img, cb) moves all 4 chunks
    act_v = act.ap().rearrange("n (jc p) m -> n p jc m", p=P)

    from contextlib import ExitStack
    with ExitStack() as ctx:
        a_sb = [ctx.enter_context(
            nc.sbuf_tensor(f"a_sb{h}", [P, NCHUNK * HW], A_DT)).ap()
            for h in range(2)]
        a_sb_v = [a.rearrange("p (jc m) -> p jc m", m=HW) for a in a_sb]
        gt_sb = ctx.enter_context(
            nc.sbuf_tensor("gt_sb", [P, NCHUNK * NKEPT * P], W_DT)).ap()
        o_sb = [[ctx.enter_context(
            nc.sbuf_tensor(f"o_sb{i}_{z}", [P, HW], OUT_DT[z])).ap()
            for z in range(NCHUNK)] for i in range(IMGS)]
        psum = [ctx.enter_context(
            nc.psum_tensor(f"ps{i}", [P, 512], mybir.dt.float32)).ap()
            for i in range(8)]

        s_gt = nc.alloc_semaphore("s_gt")
        s_ld = [[nc.alloc_semaphore(f"s_ld{h}_{cb}") for cb in range(NCB)]
                for h in range(2)]
        s_mm = nc.alloc_semaphore("s_mm")
        s_cast = {"v": nc.alloc_semaphore("s_cast_v"),
                  "a": nc.alloc_semaphore("s_cast_a")}
        s_st = {"sync": nc.alloc_semaphore("s_st_sync"),
                "scalar": nc.alloc_semaphore("s_st_sca")}
        all_sems = ([s_gt, s_mm, s_cast["v"], s_cast["a"],
                     s_st["sync"], s_st["scalar"]]
                    + [s for row in s_ld for s in row])

        def emit_load(sync, img, cb):
            # one DMA: [128 part, 4 jc, 784 cols]
            if img >= 2:
                sync.wait_ge(s_mm, TILES_PER_IMG * (img - 2)
                             + TILES_PER_CB * (cb + 1))
            sync.dma_start(
                a_sb_v[img % 2][:, :, cb * CB:(cb + 1) * CB],
                act_v[img, :, :, cb * CB:(cb + 1) * CB],
            ).then_inc(s_ld[img % 2][cb], 16)

        def emit_store(eng, ring, img, zc, h2):
            # half-image-width store: cbs {2*h2, 2*h2+1}
            e = CAST_ENG[zc]
            eng.wait_ge(s_cast[e],
                        _cnt[e][tidx(img, 2 * h2 + 1, zc, P2 - 1)])
            eng.dma_start(
                out[zc].ap()[img, :, h2 * 2 * CB:(h2 + 1) * 2 * CB],
                o_sb[img][zc][:, h2 * 2 * CB:(h2 + 1) * 2 * CB],
            ).then_inc(s_st[ring], 16)

        # Stage 0: clear semaphores, then prefetch gt + (img0, cb0) on the
        # sync queue (ordered after the clears on that queue; their sem
        # increments land while the main block is still in its barrier).
        # Sems are NOT zeroed on alloc and must not carry values across
        # executions, hence the clears.
        with nc.Block("clears", no_gpsimd_drain=True) as blk:

            @blk.sync
            def _(sync):
                for s in all_sems:
                    sync.sem_clear(s)
                sync.dma_start(gt_sb[:], gtp.ap()[:]).then_inc(s_gt, 16)
                emit_load(sync, 0, 0)

        LAST = IMGS - 1  # last image: finer stores, split across both rings

        with nc.Block("main", no_gpsimd_drain=True) as blk:

            @blk.sync
            def _(sync):
                for cb in range(1, NCB):
                    emit_load(sync, 0, cb)
                for cb in range(NCB):
                    emit_load(sync, 1, cb)
                n_store = 0
                for img in range(IMGS):
                    for h2 in range(NCB // 2):
                        if img + 2 < IMGS:
                            emit_load(sync, img + 2, 2 * h2)
                            emit_load(sync, img + 2, 2 * h2 + 1)
                        for zc in range(NCHUNK):
                            if img == LAST and CAST_ENG[zc] == "a":
                                continue  # on the scalar ring
                            emit_store(sync, "sync", img, zc, h2)
                            n_store += 1
                sync.wait_ge(s_st["sync"], 16 * n_store)

            @blk.scalar
            def _(scalar):
                n_store = 0
                for t in range(NTILES):
                    zc = zc_of(t)
                    if CAST_ENG[zc] != "a":
                        continue
                    scalar.wait_ge(s_mm, t + 1)
                    img, cb = t // TILES_PER_IMG, \
                        (t % TILES_PER_IMG) // TILES_PER_CB
                    p = cb * P2 + (t % P2)
                    scalar.copy(
                        o_sb[img][zc][:, p * PT:(p + 1) * PT],
                        psum[t % 8][:, :PT],
                    ).then_inc(s_cast["a"])
                    # last image: store each finished half-row right here so
                    # the final drain is spread across both rings
                    if (img == LAST and cb % 2 == 1 and t % P2 == P2 - 1):
                        emit_store(scalar, "scalar", img, zc, cb // 2)
                        n_store += 1
                scalar.wait_ge(s_st["scalar"], 16 * n_store)

            @blk.tensor
            def _(tensor):
                # HAM warmup on junk SBUF while the first loads land: keeps
                # the PE busy from block start so the 1.2GHz cold window is
                # spent before real matmuls begin (start=True resets bank 7
                # before its first real use)
                for _ in range(24):
                    tensor.matmul(psum[7][:, :P], a_sb[0][:, :P],
                                  a_sb[0][:, :P], start=True, stop=True)
                tensor.wait_ge(s_gt, 16)
                for img in range(IMGS):
                    for cb in range(NCB):
                        tensor.wait_ge(s_ld[img % 2][cb],
                                       16 * (img // 2 + 1))
                        for zc in range(NCHUNK):
                            for p2 in range(P2):
                                t = tidx(img, cb, zc, p2)
                                if t >= 8:
                                    e = CAST_ENG[zc]
                                    tensor.wait_ge(s_cast[e],
                                                   _cnt[e][t - 8])
                                p = cb * P2 + p2
                                for i, d in enumerate(KEPT_D):
                                    jc = (zc - d) % NCHUNK
                                    mm = tensor.matmul(
                                        psum[t % 8][:, :PT],
                                        gt_sb[:, (zc * NKEPT + i) * P:
                                              (zc * NKEPT + i + 1) * P],
                                        a_sb_v[img % 2][
                                            :, jc, p * PT:(p + 1) * PT],
                                        start=(i == 0), stop=(i == NKEPT - 1),
                                    )
                                mm.then_inc(s_mm)

            @blk.vector
            def _(vector):
                for t in range(NTILES):
                    zc = zc_of(t)
                    if CAST_ENG[zc] != "v":
                        continue
                    vector.wait_ge(s_mm, t + 1)
                    img, cb = t // TILES_PER_IMG, \
                        (t % TILES_PER_IMG) // TILES_PER_CB
                    p = cb * P2 + (t % P2)
                    vector.tensor_copy(
                        o_sb[img][zc][:, p * PT:(p + 1) * PT],
                        psum[t % 8][:, :PT],
                    ).then_inc(s_cast["v"])

    nc.compile()
    return nc


def _make_gt(inhib_kernel: np.ndarray) -> np.ndarray:
    """Packed stationary blocks: col block (zc*NKEPT+i) holds
    GTs[jc*P:(jc+1)*P, zc*P:(zc+1)*P] with jc=(zc-KEPT_D[i])%NCHUNK,
    where GTs[j, r] = g[(r + ROT - j) mod C]."""
    k = np.asarray(inhib_kernel, dtype=np.float64)
    g = np.real(np.fft.ifft(1.0 / np.fft.fft(k)))
    gts = g[(np.arange(C)[None, :] + ROT - np.arange(C)[:, None]) % C]
    gtp = np.empty((P, NCHUNK * NKEPT * P), dtype=NP_OF[W_DT])
    for zc in range(NCHUNK):
        for i, d in enumerate(KEPT_D):
            jc = (zc - d) % NCHUNK
            b = zc * NKEPT + i
            gtp[:, b * P:(b + 1) * P] = gts[jc * P:(jc + 1) * P,
                                            zc * P:(zc + 1) * P]
    return np.ascontiguousarray(gtp)


def make_in_maps(activations, inhib_kernel):
    acts = np.asarray(activations, dtype=np.float32).reshape(N, C, HW)
    acts8 = acts.astype(NP_OF[A_DT])
    gtp = _make_gt(np.asarray(inhib_kernel))
    return [
        {"act": np.ascontiguousarray(acts8[c * IMGS:(c + 1) * IMGS]),
         "gtp": gtp}
        for c in range(N_CORES)
    ]


def kernel(activations, inhib_kernel):
    acts = np.asarray(activations, dtype=np.float32)
    assert acts.shape == (N, C, H, W), acts.shape

    if "nc" not in _CACHE:
        _CACHE["nc"] = _build_nc()
    nc = _CACHE["nc"]

    in_maps = make_in_maps(acts, inhib_kernel)
    res = run_bass_kernel_spmd(nc, in_maps, core_ids=list(range(N_CORES)))
    z = np.concatenate(
        [np.concatenate([r[f"out{zc}"].astype(np.float32)
                         for zc in range(NCHUNK)], axis=1)
         for r in res.results], axis=0)
    # un-rotate: y[i] = z[(i - ROT) mod C]
    y = z[:, (np.arange(C) - ROT) % C, :]
    return y.reshape(N, C, H, W)


# revision 10
# speedup vs baseline: 1.1608x; 1.1608x over previous
"""ConvergedInhibition TRN2 kernel.

The reference computes, per pixel (n,h,w), an FFT deconvolution along the
channel axis: y = ifft(fft(x)/fft(k)).real. Since k is fixed, this is a
circular convolution with g = ifft(1/fft(k)): y[i] = sum_j g[(i-j) mod C] x[j]
— a dense CxC circulant matmul applied to every pixel. Viewing activations[n]
as a [C, H*W] matrix A_n, the problem is out_n = G @ A_n: a [512,512] x
[512,3136] matmul per image, data-parallel over 32 images across 8 cores.

Implementation choices (measured on HW):
- The deconv kernel g is concentrated in a ~224-wide circular window.
  Rotating output rows by S=288 (z[r] = y[(r+S) mod C]) aligns the support
  so only 2 of 4 K-chunks of the contraction carry mass (each output row
  keeps a 256-wide sliding window of g; truncation costs ~2e-3 rel).
  The rotation is undone by a host-side gather.
- fp8 (e3m4) I/O: |x| < 6 << 15.5 = e3m4 max, 4 mantissa bits -> ~1.34e-2
  rms rounding per side (measured end-to-end rel err 1.907e-2, HW matches
  the numpy simulation exactly). Per-zc output dtype stays configurable.
  Weights are fp16 (PE upcasts operands to FP22, mixed dtypes allowed).
- Only the 8 needed [128,128] weight blocks ship (256 KB, one DMA).
- Each dma_start occupies its HWDGE ring ~630ns regardless of size, so
  DMAs are as large as possible: one per (img, cb) on loads (400KB, 3D
  access pattern covering all 4 channel chunks), half-image-width stores.
  All loads+stores ride the sync ring; gt + the first block are prefetched
  from the semaphore-clears block so their sems fire before the main
  block's barrier lifts.
- PSUM->out casts are split between DVE (zc 0,1) and ACT (zc 2,3), each
  with its own completion semaphore (bank t%8 is always cast by the same
  engine since t and t-8 share (zc, p2)), so cast rate keeps up with PE.
"""

import numpy as np
import ml_dtypes

import concourse.bass as bass  # noqa: F401  (registers bass types)
import concourse.mybir as mybir
from concourse import bacc
from concourse.bass_utils import run_bass_kernel_spmd

N_CORES = 8
N, C, H, W = 32, 512, 56, 56
HW = H * W                      # 3136
IMGS = N // N_CORES             # 4 images per core
P = 128                         # partitions
NCHUNK = C // P                 # 4
PT = 392                        # pixel tile (free dim), 3136 = 8*392
NPT = HW // PT                  # 8
CB = 784                        # column block, 3136 = 4*784
NCB = HW // CB                  # 4
ROT = 288                       # output-row rotation aligning g's support
KEPT_D = (0, 1)                 # kept (zc - jc) mod 4 chunk distances
NKEPT = len(KEPT_D)

F8 = mybir.dt.float8e3
F16 = mybir.dt.float16
NP_OF = {F8: ml_dtypes.float8_e3m4, F16: np.float16}
A_DT = F8                       # activation (input) dtype
OUT_DT = [F8, F8, F8, F8]       # output dtype per output chunk zc
W_DT = F16                      # gt weights
CAST_ENG = ["v", "v", "a", "a"]  # cast engine per zc (v=DVE, a=ACT)

P2 = NPT // NCB                       # p-tiles per column block (2)
TILES_PER_CB = NCHUNK * P2            # 8 psum tiles per (img, cb)
TILES_PER_IMG = NCB * TILES_PER_CB    # 32
NTILES = IMGS * TILES_PER_IMG         # 128


def tidx(img, cb, zc, p2):
    return img * TILES_PER_IMG + cb * TILES_PER_CB + zc * P2 + p2


def zc_of(t):
    return (t % TILES_PER_CB) // P2


# cnt_eng[e][t]: number of tiles t' <= t whose cast runs on engine e
_cnt = {"v": [0] * NTILES, "a": [0] * NTILES}
_c = {"v": 0, "a": 0}
for _t in range(NTILES):
    _c[CAST_ENG[zc_of(_t)]] += 1
    _cnt["v"][_t] = _c["v"]
    _cnt["a"][_t] = _c["a"]

_CACHE = {}


def _build_nc():
    nc = bacc.Bacc("TRN2", target_bir_lowering=False, debug=False,
                   num_devices=N_CORES)
    act = nc.dram_tensor("act", [IMGS, C, HW], A_DT, kind="ExternalInput")
    gtp = nc.dram_tensor("gtp", [P, NCHUNK * NKEPT * P], W_DT,
                         kind="ExternalInput")
    out = [nc.dram_tensor(f"out{zc}", [IMGS, P, HW], OUT_DT[zc],
                          kind="ExternalOutput") for zc in range(NCHUNK)]

    # [img, p, jc, m]: partition-major view of the (jc p) channel split so
    # one DMA per (img, cb) moves all 4 chunks
    act_v = act.ap().rearrange("n (jc p) m -> n p jc m", p=P)

    from contextlib import ExitStack
    with ExitStack() as ctx:
        a_sb = [ctx.enter_context(
            nc.sbuf_tensor(f"a_sb{h}", [P, NCHUNK * HW], A_DT)).ap()
            for h in range(2)]
        a_sb_v = [a.rearrange("p (jc m) -> p jc m", m=HW) for a in a_sb]
        gt_sb = ctx.enter_context(
            nc.sbuf_tensor("gt_sb", [P, NCHUNK * NKEPT * P], W_DT)).ap()
        o_sb = [[ctx.enter_context(
            nc.sbuf_tensor(f"o_sb{i}_{z}", [P, HW], OUT_DT[z])).ap()
            for z in range(NCHUNK)] for i in range(IMGS)]
        psum = [ctx.enter_context(
            nc.psum_tensor(f"ps{i}", [P, 512], mybir.dt.float32)).ap()
            for i in range(8)]

        s_gt = nc.alloc_semaphore("s_gt")
        s_ld = [[nc.alloc_semaphore(f"s_ld{h}_{cb}") for cb in range(NCB)]
                for h in range(2)]
        s_mm = nc.alloc_semaphore("s_mm")
        s_cast = {"v": nc.alloc_semaphore("s_cast_v"),
                  "a": nc.alloc_semaphore("s_cast_a")}
        s_st = {"sync": nc.alloc_semaphore("s_st_sync"),
                "scalar": nc.alloc_semaphore("s_st_sca")}
        all_sems = ([s_gt, s_mm, s_cast["v"], s_cast["a"],
                     s_st["sync"], s_st["scalar"]]
                    + [s for row in s_ld for s in row])

        def emit_load(sync, img, cb):
            # one DMA: [128 part, 4 jc, 784 cols]
            if img >= 2:
                sync.wait_ge(s_mm, TILES_PER_IMG * (img - 2)
                             + TILES_PER_CB * (cb + 1))
            sync.dma_start(
                a_sb_v[img % 2][:, :, cb * CB:(cb + 1) * CB],
                act_v[img, :, :, cb * CB:(cb + 1) * CB],
            ).then_inc(s_ld[img % 2][cb], 16)

        def emit_store(eng, ring, img, zc, h2):
            # half-image-width store: cbs {2*h2, 2*h2+1}
            e = CAST_ENG[zc]
            eng.wait_ge(s_cast[e],
                        _cnt[e][tidx(img, 2 * h2 + 1, zc, P2 - 1)])
            eng.dma_start(
                out[zc].ap()[img, :, h2 * 2 * CB:(h2 + 1) * 2 * CB],
                o_sb[img][zc][:, h2 * 2 * CB:(h2 + 1) * 2 * CB],
            ).then_inc(s_st[ring], 16)

        # Stage 0: clear semaphores, then prefetch gt + (img0, cb0) on the
        # sync queue (ordered after the clears on that queue; their sem
        # increments land while the main block is still in its barrier).
        # Sems are NOT zeroed on alloc and must not carry values across
        # executions, hence the clears.
        with nc.Block("clears") as blk:

            @blk.sync
            def _(sync):
                for s in all_sems:
                    sync.sem_clear(s)
                sync.dma_start(gt_sb[:], gtp.ap()[:]).then_inc(s_gt, 16)
                emit_load(sync, 0, 0)

        LAST = IMGS - 1  # last image: finer stores, split across both rings

        with nc.Block("main") as blk:

            @blk.sync
            def _(sync):
                for cb in range(1, NCB):
                    emit_load(sync, 0, cb)
                for cb in range(NCB):
                    emit_load(sync, 1, cb)
                n_store = 0
                for img in range(IMGS):
                    for h2 in range(NCB // 2):
                        if img + 2 < IMGS:
                            emit_load(sync, img + 2, 2 * h2)
                            emit_load(sync, img + 2, 2 * h2 + 1)
                        for zc in range(NCHUNK):
                            if img == LAST and CAST_ENG[zc] == "a":
                                continue  # on the scalar ring
                            emit_store(sync, "sync", img, zc, h2)
                            n_store += 1
                sync.wait_ge(s_st["sync"], 16 * n_store)

            @blk.scalar
            def _(scalar):
                n_store = 0
                for t in range(NTILES):
                    zc = zc_of(t)
                    if CAST_ENG[zc] != "a":
                        continue
                    scalar.wait_ge(s_mm, t + 1)
                    img, cb = t // TILES_PER_IMG, \
                        (t % TILES_PER_IMG) // TILES_PER_CB
                    p = cb * P2 + (t % P2)
                    scalar.copy(
                        o_sb[img][zc][:, p * PT:(p + 1) * PT],
                        psum[t % 8][:, :PT],
                    ).then_inc(s_cast["a"])
                    # last image: store each finished half-row right here so
                    # the final drain is spread across both rings
                    if (img == LAST and cb % 2 == 1 and t % P2 == P2 - 1):
                        emit_store(scalar, "scalar", img, zc, cb // 2)
                        n_store += 1
                scalar.wait_ge(s_st["scalar"], 16 * n_store)

            @blk.tensor
            def _(tensor):
                # HAM warmup on junk SBUF while the first loads land: keeps
                # the PE busy from block start so the 1.2GHz cold window is
                # spent before real matmuls begin (start=True resets bank 7
                # before its first real use)
                for _ in range(26):
                    tensor.matmul(psum[7][:, :P], a_sb[0][:, :P],
                                  a_sb[0][:, :P], start=True, stop=True)
                tensor.wait_ge(s_gt, 16)
                for img in range(IMGS):
                    for cb in range(NCB):
                        tensor.wait_ge(s_ld[img % 2][cb],
                                       16 * (img // 2 + 1))
                        for zc in range(NCHUNK):
                            for p2 in range(P2):
                                t = tidx(img, cb, zc, p2)
                                if t >= 8:
                                    e = CAST_ENG[zc]
                                    tensor.wait_ge(s_cast[e],
                                                   _cnt[e][t - 8])
                                p = cb * P2 + p2
                                for i, d in enumerate(KEPT_D):
                                    jc = (zc - d) % NCHUNK
                                    mm = tensor.matmul(
                                        psum[t % 8][:, :PT],
                                        gt_sb[:, (zc * NKEPT + i) * P:
                                              (zc * NKEPT + i + 1) * P],
                                        a_sb_v[img % 2][
                                            :, jc, p * PT:(p + 1) * PT],
                                        start=(i == 0), stop=(i == NKEPT - 1),
                                    )
                                mm.then_inc(s_mm)

            @blk.vector
            def _(vector):
                for t in range(NTILES):
                    zc = zc_of(t)
                    if CAST_ENG[zc] != "v":
                        continue
                    vector.wait_ge(s_mm, t + 1)
                    img, cb = t // TILES_PER_IMG, \
                        (t % TILES_PER_IMG) // TILES_PER_CB
                    p = cb * P2 + (t % P2)
                    vector.tensor_copy(
                        o_sb[img][zc][:, p * PT:(p + 1) * PT],
                        psum[t % 8][:, :PT],
                    ).then_inc(s_cast["v"])

    nc.compile()
    return nc


def _make_gt(inhib_kernel: np.ndarray) -> np.ndarray:
    """Packed stationary blocks: col block (zc*NKEPT+i) holds
    GTs[jc*P:(jc+1)*P, zc*P:(zc+1)*P] with jc=(zc-KEPT_D[i])%NCHUNK,
    where GTs[j, r] = g[(r + ROT - j) mod C]."""
    k = np.asarray(inhib_kernel, dtype=np.float64)
    g = np.real(np.fft.ifft(1.0 / np.fft.fft(k)))
    gts = g[(np.arange(C)[None, :] + ROT - np.arange(C)[:, None]) % C]
    gtp = np.empty((P, NCHUNK * NKEPT * P), dtype=NP_OF[W_DT])
    for zc in range(NCHUNK):
        for i, d in enumerate(KEPT_D):
            jc = (zc - d) % NCHUNK
            b = zc * NKEPT + i
            gtp[:, b * P:(b + 1) * P] = gts[jc * P:(jc + 1) * P,
                                            zc * P:(zc + 1) * P]
    return np.ascontiguousarray(gtp)


def make_in_maps(activations, inhib_kernel):
    acts = np.asarray(activations, dtype=np.float32).reshape(N, C, HW)
    acts8 = acts.astype(NP_OF[A_DT])
    gtp = _make_gt(np.asarray(inhib_kernel))
    return [
        {"act": np.ascontiguousarray(acts8[c * IMGS:(c + 1) * IMGS]),
         "gtp": gtp}
        for c in range(N_CORES)
    ]


def kernel(activations, inhib_kernel):
    acts = np.asarray(activations, dtype=np.float32)
    assert acts.shape == (N, C, H, W), acts.shape

    if "nc" not in _CACHE:
        _CACHE["nc"] = _build_nc()
    nc = _CACHE["nc"]

    in_maps = make_in_maps(acts, inhib_kernel)
    res = run_bass_kernel_spmd(nc, in_maps, core_ids=list(range(N_CORES)))
    z = np.concatenate(
        [np.concatenate([r[f"out{zc}"].astype(np.float32)
                         for zc in range(NCHUNK)], axis=1)
         for r in res.results], axis=0)
    # un-rotate: y[i] = z[(i - ROT) mod C]
    y = z[:, (np.arange(C) - ROT) % C, :]
    return y.reshape(N, C, H, W)


# revision 14
# speedup vs baseline: 1.1609x; 1.0001x over previous
"""ConvergedInhibition TRN2 kernel.

The reference computes, per pixel (n,h,w), an FFT deconvolution along the
channel axis: y = ifft(fft(x)/fft(k)).real. Since k is fixed, this is a
circular convolution with g = ifft(1/fft(k)): y[i] = sum_j g[(i-j) mod C] x[j]
— a dense CxC circulant matmul applied to every pixel. Viewing activations[n]
as a [C, H*W] matrix A_n, the problem is out_n = G @ A_n: a [512,512] x
[512,3136] matmul per image, data-parallel over 32 images across 8 cores.

Implementation choices (measured on HW):
- The deconv kernel g is concentrated in a ~224-wide circular window.
  Rotating output rows by S=288 (z[r] = y[(r+S) mod C]) aligns the support
  so only 2 of 4 K-chunks of the contraction carry mass (each output row
  keeps a 256-wide sliding window of g; truncation costs ~2e-3 rel).
  The rotation is undone by a host-side gather.
- fp8 (e3m4) I/O: |x| < 6 << 15.5 = e3m4 max, 4 mantissa bits -> ~1.34e-2
  rms rounding per side (measured end-to-end rel err 1.907e-2, HW matches
  the numpy simulation exactly). Per-zc output dtype stays configurable.
  Weights are fp16 (PE upcasts operands to FP22, mixed dtypes allowed).
- Only the 8 needed [128,128] weight blocks ship (256 KB, one DMA).
- Each dma_start occupies its HWDGE ring ~630ns regardless of size, so
  DMAs are as large as possible: one per (img, cb) on loads (400KB, 3D
  access pattern covering all 4 channel chunks), half-image-width stores.
  All loads+stores ride the sync ring; gt + the first block are prefetched
  from the semaphore-clears block so their sems fire before the main
  block's barrier lifts.
- PSUM->out casts are split between DVE (zc 0,1) and ACT (zc 2,3), each
  with its own completion semaphore (bank t%8 is always cast by the same
  engine since t and t-8 share (zc, p2)), so cast rate keeps up with PE.
"""

import numpy as np
import ml_dtypes

import concourse.bass as bass  # noqa: F401  (registers bass types)
import concourse.mybir as mybir
from concourse import bacc
from concourse.bass_utils import run_bass_kernel_spmd

N_CORES = 8
N, C, H, W = 32, 512, 56, 56
HW = H * W                      # 3136
IMGS = N // N_CORES             # 4 images per core
P = 128                         # partitions
NCHUNK = C // P                 # 4
PT = 392                        # pixel tile (free dim), 3136 = 8*392
NPT = HW // PT                  # 8
CB = 784                        # column block, 3136 = 4*784
NCB = HW // CB                  # 4
ROT = 288                       # output-row rotation aligning g's support
KEPT_D = (0, 1)                 # kept (zc - jc) mod 4 chunk distances
NKEPT = len(KEPT_D)

F8 = mybir.dt.float8e3
F16 = mybir.dt.float16
NP_OF = {F8: ml_dtypes.float8_e3m4, F16: np.float16}
A_DT = F8                       # activation (input) dtype
OUT_DT = [F8, F8, F8, F8]       # output dtype per output chunk zc
W_DT = F16                      # gt weights
CAST_ENG = ["v", "v", "a", "a"]  # cast engine per zc (v=DVE, a=ACT)

P2 = NPT // NCB                       # p-tiles per column block (2)
TILES_PER_CB = NCHUNK * P2            # 8 psum tiles per (img, cb)
TILES_PER_IMG = NCB * TILES_PER_CB    # 32
NTILES = IMGS * TILES_PER_IMG         # 128


def tidx(img, cb, zc, p2):
    return img * TILES_PER_IMG + cb * TILES_PER_CB + zc * P2 + p2


def zc_of(t):
    return (t % TILES_PER_CB) // P2


# cnt_eng[e][t]: number of tiles t' <= t whose cast runs on engine e
_cnt = {"v": [0] * NTILES, "a": [0] * NTILES}
_c = {"v": 0, "a": 0}
for _t in range(NTILES):
    _c[CAST_ENG[zc_of(_t)]] += 1
    _cnt["v"][_t] = _c["v"]
    _cnt["a"][_t] = _c["a"]

_CACHE = {}


def _build_nc():
    nc = bacc.Bacc("TRN2", target_bir_lowering=False, debug=False,
                   num_devices=N_CORES)
    act = nc.dram_tensor("act", [IMGS, C, HW], A_DT, kind="ExternalInput")
    gtp = nc.dram_tensor("gtp", [P, NCHUNK * NKEPT * P], W_DT,
                         kind="ExternalInput")
    out = [nc.dram_tensor(f"out{zc}", [IMGS, P, HW], OUT_DT[zc],
                          kind="ExternalOutput") for zc in range(NCHUNK)]

    # [img, p, jc, m]: partition-major view of the (jc p) channel split so
    # one DMA per (img, cb) moves all 4 chunks
    act_v = act.ap().rearrange("n (jc p) m -> n p jc m", p=P)

    from contextlib import ExitStack
    with ExitStack() as ctx:
        a_sb = [ctx.enter_context(
            nc.sbuf_tensor(f"a_sb{h}", [P, NCHUNK * HW], A_DT)).ap()
            for h in range(2)]
        a_sb_v = [a.rearrange("p (jc m) -> p jc m", m=HW) for a in a_sb]
        gt_sb = ctx.enter_context(
            nc.sbuf_tensor("gt_sb", [P, NCHUNK * NKEPT * P], W_DT)).ap()
        o_sb = [[ctx.enter_context(
            nc.sbuf_tensor(f"o_sb{i}_{z}", [P, HW], OUT_DT[z])).ap()
            for z in range(NCHUNK)] for i in range(IMGS)]
        psum = [ctx.enter_context(
            nc.psum_tensor(f"ps{i}", [P, 512], mybir.dt.float32)).ap()
            for i in range(8)]

        s_gt = nc.alloc_semaphore("s_gt")
        s_ld = [[nc.alloc_semaphore(f"s_ld{h}_{cb}") for cb in range(NCB)]
                for h in range(2)]
        s_mm = nc.alloc_semaphore("s_mm")
        s_cast = {"v": nc.alloc_semaphore("s_cast_v"),
                  "a": nc.alloc_semaphore("s_cast_a")}
        s_st = {"sync": nc.alloc_semaphore("s_st_sync"),
                "scalar": nc.alloc_semaphore("s_st_sca")}
        all_sems = ([s_gt, s_mm, s_cast["v"], s_cast["a"],
                     s_st["sync"], s_st["scalar"]]
                    + [s for row in s_ld for s in row])

        def emit_load(sync, img, cb):
            # one DMA: [128 part, 4 jc, 784 cols]
            if img >= 2:
                sync.wait_ge(s_mm, TILES_PER_IMG * (img - 2)
                             + TILES_PER_CB * (cb + 1))
            sync.dma_start(
                a_sb_v[img % 2][:, :, cb * CB:(cb + 1) * CB],
                act_v[img, :, :, cb * CB:(cb + 1) * CB],
            ).then_inc(s_ld[img % 2][cb], 16)

        def emit_store(eng, ring, img, zc, h2):
            # half-image-width store: cbs {2*h2, 2*h2+1}
            e = CAST_ENG[zc]
            eng.wait_ge(s_cast[e],
                        _cnt[e][tidx(img, 2 * h2 + 1, zc, P2 - 1)])
            eng.dma_start(
                out[zc].ap()[img, :, h2 * 2 * CB:(h2 + 1) * 2 * CB],
                o_sb[img][zc][:, h2 * 2 * CB:(h2 + 1) * 2 * CB],
            ).then_inc(s_st[ring], 16)

        # Stage 0: clear semaphores, then prefetch gt + (img0, cb0) on the
        # sync queue (ordered after the clears on that queue; their sem
        # increments land while the main block is still in its barrier).
        # Sems are NOT zeroed on alloc and must not carry values across
        # executions, hence the clears.
        with nc.Block("clears") as blk:

            @blk.sync
            def _(sync):
                for s in all_sems:
                    sync.sem_clear(s)
                sync.dma_start(gt_sb[:], gtp.ap()[:]).then_inc(s_gt, 16)
                emit_load(sync, 0, 0)

            @blk.tensor
            def _(tensor):
                # HAM warmup on junk SBUF while the clears + first loads
                # issue: the PE's clock gate needs ~3.4us of sustained
                # activity to lift the 1.2GHz cold throttle, so burn that
                # window here where the tensor engine would idle anyway
                # (start=True resets bank 7 before its first real use).
                for _ in range(30):
                    tensor.matmul(psum[7][:, :P], a_sb[0][:, :P],
                                  a_sb[0][:, :P], start=True, stop=True)

        LAST = IMGS - 1  # last image: finer stores, split across both rings

        with nc.Block("main") as blk:

            @blk.sync
            def _(sync):
                for cb in range(1, NCB):
                    emit_load(sync, 0, cb)
                for cb in range(NCB):
                    emit_load(sync, 1, cb)
                n_store = 0
                for img in range(IMGS):
                    for h2 in range(NCB // 2):
                        if img + 2 < IMGS:
                            emit_load(sync, img + 2, 2 * h2)
                            emit_load(sync, img + 2, 2 * h2 + 1)
                        for zc in range(NCHUNK):
                            if img == LAST and CAST_ENG[zc] == "a":
                                continue  # on the scalar ring
                            if img == LAST and h2 == 1:
                                continue  # finer drain stores below
                            emit_store(sync, "sync", img, zc, h2)
                            n_store += 1
                # drain: per-column-block stores for the last image's final
                # half so the closing transfer+receipt is as short as possible
                for cb in (2, 3):
                    for zc in range(NCHUNK):
                        if CAST_ENG[zc] != "v":
                            continue
                        e = CAST_ENG[zc]
                        sync.wait_ge(s_cast[e],
                                     _cnt[e][tidx(LAST, cb, zc, P2 - 1)])
                        sync.dma_start(
                            out[zc].ap()[LAST, :, cb * CB:(cb + 1) * CB],
                            o_sb[LAST][zc][:, cb * CB:(cb + 1) * CB],
                        ).then_inc(s_st["sync"], 16)
                        n_store += 1
                sync.wait_ge(s_st["sync"], 16 * n_store)

            @blk.scalar
            def _(scalar):
                n_store = 0
                for t in range(NTILES):
                    zc = zc_of(t)
                    if CAST_ENG[zc] != "a":
                        continue
                    scalar.wait_ge(s_mm, t + 1)
                    img, cb = t // TILES_PER_IMG, \
                        (t % TILES_PER_IMG) // TILES_PER_CB
                    p = cb * P2 + (t % P2)
                    scalar.copy(
                        o_sb[img][zc][:, p * PT:(p + 1) * PT],
                        psum[t % 8][:, :PT],
                    ).then_inc(s_cast["a"])
                    # last image, first half: store right here (slack in the
                    # cast chain); the final half's stores are deferred below
                    # so the last casts aren't delayed by store issue
                    if (img == LAST and cb == 1 and t % P2 == P2 - 1):
                        emit_store(scalar, "scalar", img, zc, 0)
                        n_store += 1
                # drain: last-half stores of the last image, per column
                # block, after all casts (waits are already satisfied)
                for cb in (2, 3):
                    for zc in range(NCHUNK):
                        if CAST_ENG[zc] != "a":
                            continue
                        scalar.dma_start(
                            out[zc].ap()[LAST, :, cb * CB:(cb + 1) * CB],
                            o_sb[LAST][zc][:, cb * CB:(cb + 1) * CB],
                        ).then_inc(s_st["scalar"], 16)
                        n_store += 1
                scalar.wait_ge(s_st["scalar"], 16 * n_store)

            @blk.tensor
            def _(tensor):
                tensor.wait_ge(s_gt, 16)
                for img in range(IMGS):
                    for cb in range(NCB):
                        tensor.wait_ge(s_ld[img % 2][cb],
                                       16 * (img // 2 + 1))
                        for zc in range(NCHUNK):
                            for p2 in range(P2):
                                t = tidx(img, cb, zc, p2)
                                if t >= 8:
                                    e = CAST_ENG[zc]
                                    tensor.wait_ge(s_cast[e],
                                                   _cnt[e][t - 8])
                                p = cb * P2 + p2
                                for i, d in enumerate(KEPT_D):
                                    jc = (zc - d) % NCHUNK
                                    mm = tensor.matmul(
                                        psum[t % 8][:, :PT],
                                        gt_sb[:, (zc * NKEPT + i) * P:
                                              (zc * NKEPT + i + 1) * P],
                                        a_sb_v[img % 2][
                                            :, jc, p * PT:(p + 1) * PT],
                                        start=(i == 0), stop=(i == NKEPT - 1),
                                    )
                                mm.then_inc(s_mm)

            @blk.vector
            def _(vector):
                for t in range(NTILES):
                    zc = zc_of(t)
                    if CAST_ENG[zc] != "v":
                        continue
                    vector.wait_ge(s_mm, t + 1)
                    img, cb = t // TILES_PER_IMG, \
                        (t % TILES_PER_IMG) // TILES_PER_CB
                    p = cb * P2 + (t % P2)
                    vector.tensor_copy(
                        o_sb[img][zc][:, p * PT:(p + 1) * PT],
                        psum[t % 8][:, :PT],
                    ).then_inc(s_cast["v"])

    nc.compile()
    return nc


def _make_gt(inhib_kernel: np.ndarray) -> np.ndarray:
    """Packed stationary blocks: col block (zc*NKEPT+i) holds
    GTs[jc*P:(jc+1)*P, zc*P:(zc+1)*P] with jc=(zc-KEPT_D[i])%NCHUNK,
    where GTs[j, r] = g[(r + ROT - j) mod C]."""
    k = np.asarray(inhib_kernel, dtype=np.float64)
    g = np.real(np.fft.ifft(1.0 / np.fft.fft(k)))
    gts = g[(np.arange(C)[None, :] + ROT - np.arange(C)[:, None]) % C]
    gtp = np.empty((P, NCHUNK * NKEPT * P), dtype=NP_OF[W_DT])
    for zc in range(NCHUNK):
        for i, d in enumerate(KEPT_D):
            jc = (zc - d) % NCHUNK
            b = zc * NKEPT + i
            gtp[:, b * P:(b + 1) * P] = gts[jc * P:(jc + 1) * P,
                                            zc * P:(zc + 1) * P]
    return np.ascontiguousarray(gtp)


def make_in_maps(activations, inhib_kernel):
    acts = np.asarray(activations, dtype=np.float32).reshape(N, C, HW)
    acts8 = acts.astype(NP_OF[A_DT])
    gtp = _make_gt(np.asarray(inhib_kernel))
    return [
        {"act": np.ascontiguousarray(acts8[c * IMGS:(c + 1) * IMGS]),
         "gtp": gtp}
        for c in range(N_CORES)
    ]


def kernel(activations, inhib_kernel):
    acts = np.asarray(activations, dtype=np.float32)
    assert acts.shape == (N, C, H, W), acts.shape

    if "nc" not in _CACHE:
        _CACHE["nc"] = _build_nc()
    nc = _CACHE["nc"]

    in_maps = make_in_maps(acts, inhib_kernel)
    res = run_bass_kernel_spmd(nc, in_maps, core_ids=list(range(N_CORES)))
    z = np.concatenate(
        [np.concatenate([r[f"out{zc}"].astype(np.float32)
                         for zc in range(NCHUNK)], axis=1)
         for r in res.results], axis=0)
    # un-rotate: y[i] = z[(i - ROT) mod C]
    y = z[:, (np.arange(C) - ROT) % C, :]
    return y.reshape(N, C, H, W)


# revision 16
# speedup vs baseline: 1.1884x; 1.0237x over previous
"""ConvergedInhibition TRN2 kernel.

The reference computes, per pixel (n,h,w), an FFT deconvolution along the
channel axis: y = ifft(fft(x)/fft(k)).real. Since k is fixed, this is a
circular convolution with g = ifft(1/fft(k)): y[i] = sum_j g[(i-j) mod C] x[j]
— a dense CxC circulant matmul applied to every pixel. Viewing activations[n]
as a [C, H*W] matrix A_n, the problem is out_n = G @ A_n: a [512,512] x
[512,3136] matmul per image, data-parallel over 32 images across 8 cores.

Implementation choices (measured on HW):
- The deconv kernel g is concentrated in a ~224-wide circular window.
  Rotating output rows by S=288 (z[r] = y[(r+S) mod C]) aligns the support
  so only 2 of 4 K-chunks of the contraction carry mass (each output row
  keeps a 256-wide sliding window of g; truncation costs ~2e-3 rel).
  The rotation is undone by a host-side gather.
- fp8 (e3m4) I/O: |x| < 6 << 15.5 = e3m4 max, 4 mantissa bits -> ~1.34e-2
  rms rounding per side (measured end-to-end rel err 1.907e-2, HW matches
  the numpy simulation exactly). Per-zc output dtype stays configurable.
  Weights are fp16 (PE upcasts operands to FP22, mixed dtypes allowed).
- Only the 8 needed [128,128] weight blocks ship (256 KB, one DMA).
- Each dma_start occupies its HWDGE ring ~630ns regardless of size, so
  DMAs are as large as possible: one per (img, cb) on loads (400KB, 3D
  access pattern covering all 4 channel chunks), half-image-width stores.
  All loads+stores ride the sync ring; gt + the first block are prefetched
  from the semaphore-clears block so their sems fire before the main
  block's barrier lifts.
- PSUM->out casts are split between DVE (zc 0,1) and ACT (zc 2,3), each
  with its own completion semaphore (bank t%8 is always cast by the same
  engine since t and t-8 share (zc, p2)), so cast rate keeps up with PE.
"""

import numpy as np
import ml_dtypes

import concourse.bass as bass  # noqa: F401  (registers bass types)
import concourse.mybir as mybir
from concourse import bacc
from concourse.bass_utils import run_bass_kernel_spmd

N_CORES = 8
N, C, H, W = 32, 512, 56, 56
HW = H * W                      # 3136
IMGS = N // N_CORES             # 4 images per core
P = 128                         # partitions
NCHUNK = C // P                 # 4
PT = 392                        # pixel tile (free dim), 3136 = 8*392
NPT = HW // PT                  # 8
CB = 784                        # column block, 3136 = 4*784
NCB = HW // CB                  # 4
ROT = 288                       # output-row rotation aligning g's support
KEPT_D = (0, 1)                 # kept (zc - jc) mod 4 chunk distances
NKEPT = len(KEPT_D)

F8 = mybir.dt.float8e3
F16 = mybir.dt.float16
NP_OF = {F8: ml_dtypes.float8_e3m4, F16: np.float16}
A_DT = F8                       # activation (input) dtype
OUT_DT = [F8, F8, F8, F8]       # output dtype per output chunk zc
W_DT = F16                      # gt weights
CAST_ENG = ["v", "v", "a", "a"]  # cast engine per zc (v=DVE, a=ACT)

P2 = NPT // NCB                       # p-tiles per column block (2)
TILES_PER_CB = NCHUNK * P2            # 8 psum tiles per (img, cb)
TILES_PER_IMG = NCB * TILES_PER_CB    # 32
NTILES = IMGS * TILES_PER_IMG         # 128


def tidx(img, cb, zc, p2):
    return img * TILES_PER_IMG + cb * TILES_PER_CB + zc * P2 + p2


def zc_of(t):
    return (t % TILES_PER_CB) // P2


# cnt_eng[e][t]: number of tiles t' <= t whose cast runs on engine e
_cnt = {"v": [0] * NTILES, "a": [0] * NTILES}
_c = {"v": 0, "a": 0}
for _t in range(NTILES):
    _c[CAST_ENG[zc_of(_t)]] += 1
    _cnt["v"][_t] = _c["v"]
    _cnt["a"][_t] = _c["a"]

_CACHE = {}


def _build_nc():
    nc = bacc.Bacc("TRN2", target_bir_lowering=False, debug=False,
                   num_devices=N_CORES)
    act = nc.dram_tensor("act", [IMGS, C, HW], A_DT, kind="ExternalInput")
    gtp = nc.dram_tensor("gtp", [P, NCHUNK * NKEPT * P], W_DT,
                         kind="ExternalInput")
    out = [nc.dram_tensor(f"out{zc}", [IMGS, P, HW], OUT_DT[zc],
                          kind="ExternalOutput") for zc in range(NCHUNK)]

    # [img, p, jc, m]: partition-major view of the (jc p) channel split so
    # one DMA per (img, cb) moves all 4 chunks
    act_v = act.ap().rearrange("n (jc p) m -> n p jc m", p=P)

    from contextlib import ExitStack
    with ExitStack() as ctx:
        a_sb = [ctx.enter_context(
            nc.sbuf_tensor(f"a_sb{h}", [P, NCHUNK * HW], A_DT)).ap()
            for h in range(2)]
        a_sb_v = [a.rearrange("p (jc m) -> p jc m", m=HW) for a in a_sb]
        gt_sb = ctx.enter_context(
            nc.sbuf_tensor("gt_sb", [P, NCHUNK * NKEPT * P], W_DT)).ap()
        o_sb = [[ctx.enter_context(
            nc.sbuf_tensor(f"o_sb{i}_{z}", [P, HW], OUT_DT[z])).ap()
            for z in range(NCHUNK)] for i in range(IMGS)]
        psum = [ctx.enter_context(
            nc.psum_tensor(f"ps{i}", [P, 512], mybir.dt.float32)).ap()
            for i in range(8)]

        s_gt = nc.alloc_semaphore("s_gt")
        s_ld = [[nc.alloc_semaphore(f"s_ld{h}_{cb}") for cb in range(NCB)]
                for h in range(2)]
        s_mm = nc.alloc_semaphore("s_mm")
        s_cast = {"v": nc.alloc_semaphore("s_cast_v"),
                  "a": nc.alloc_semaphore("s_cast_a")}
        s_st = {"sync": nc.alloc_semaphore("s_st_sync"),
                "scalar": nc.alloc_semaphore("s_st_sca")}
        all_sems = ([s_gt, s_mm, s_cast["v"], s_cast["a"],
                     s_st["sync"], s_st["scalar"]]
                    + [s for row in s_ld for s in row])

        def emit_load(sync, img, cb):
            # one DMA: [128 part, 4 jc, 784 cols]
            if img >= 2:
                sync.wait_ge(s_mm, TILES_PER_IMG * (img - 2)
                             + TILES_PER_CB * (cb + 1))
            sync.dma_start(
                a_sb_v[img % 2][:, :, cb * CB:(cb + 1) * CB],
                act_v[img, :, :, cb * CB:(cb + 1) * CB],
            ).then_inc(s_ld[img % 2][cb], 16)

        def emit_store(eng, ring, img, zc, h2):
            # half-image-width store: cbs {2*h2, 2*h2+1}
            e = CAST_ENG[zc]
            eng.wait_ge(s_cast[e],
                        _cnt[e][tidx(img, 2 * h2 + 1, zc, P2 - 1)])
            eng.dma_start(
                out[zc].ap()[img, :, h2 * 2 * CB:(h2 + 1) * 2 * CB],
                o_sb[img][zc][:, h2 * 2 * CB:(h2 + 1) * 2 * CB],
            ).then_inc(s_st[ring], 16)

        # Stage 0: clear semaphores, then prefetch gt + (img0, cb0) on the
        # sync queue (ordered after the clears on that queue; their sem
        # increments land while the main block is still in its barrier).
        # Sems are NOT zeroed on alloc and must not carry values across
        # executions, hence the clears.
        with nc.Block("clears") as blk:

            @blk.sync
            def _(sync):
                for s in all_sems:
                    sync.sem_clear(s)
                emit_load(sync, 0, 0)

            @blk.scalar
            def _(scalar):
                # gt rides the otherwise-idle scalar ring so its receipt
                # doesn't queue behind the activation packets
                scalar.dma_start(gt_sb[:], gtp.ap()[:]).then_inc(s_gt, 16)

            @blk.tensor
            def _(tensor):
                # HAM warmup on junk SBUF while the clears + first loads
                # issue: the PE's clock gate needs ~3.4us of sustained
                # activity to lift the 1.2GHz cold throttle, so burn that
                # window here where the tensor engine would idle anyway
                # (start=True resets bank 7 before its first real use).
                for _ in range(28):
                    tensor.matmul(psum[7][:, :P], a_sb[0][:, :P],
                                  a_sb[0][:, :P], start=True, stop=True)

        LAST = IMGS - 1  # last image: finer stores, split across both rings

        with nc.Block("main") as blk:

            @blk.sync
            def _(sync):
                for cb in range(1, NCB):
                    emit_load(sync, 0, cb)
                for cb in range(NCB):
                    emit_load(sync, 1, cb)
                n_store = 0
                for img in range(IMGS):
                    for h2 in range(NCB // 2):
                        if img + 2 < IMGS:
                            emit_load(sync, img + 2, 2 * h2)
                            emit_load(sync, img + 2, 2 * h2 + 1)
                        for zc in range(NCHUNK):
                            if img == LAST and CAST_ENG[zc] == "a":
                                continue  # on the scalar ring
                            if img == LAST and h2 == 1:
                                continue  # finer drain stores below
                            emit_store(sync, "sync", img, zc, h2)
                            n_store += 1
                # drain: per-column-block stores for the last image's final
                # half so the closing transfer+receipt is as short as possible
                for cb in (2, 3):
                    for zc in range(NCHUNK):
                        if CAST_ENG[zc] != "v":
                            continue
                        e = CAST_ENG[zc]
                        sync.wait_ge(s_cast[e],
                                     _cnt[e][tidx(LAST, cb, zc, P2 - 1)])
                        sync.dma_start(
                            out[zc].ap()[LAST, :, cb * CB:(cb + 1) * CB],
                            o_sb[LAST][zc][:, cb * CB:(cb + 1) * CB],
                        ).then_inc(s_st["sync"], 16)
                        n_store += 1
                sync.wait_ge(s_st["sync"], 16 * n_store)

            @blk.scalar
            def _(scalar):
                n_store = 0
                for t in range(NTILES):
                    zc = zc_of(t)
                    if CAST_ENG[zc] != "a":
                        continue
                    scalar.wait_ge(s_mm, t + 1)
                    img, cb = t // TILES_PER_IMG, \
                        (t % TILES_PER_IMG) // TILES_PER_CB
                    p = cb * P2 + (t % P2)
                    scalar.copy(
                        o_sb[img][zc][:, p * PT:(p + 1) * PT],
                        psum[t % 8][:, :PT],
                    ).then_inc(s_cast["a"])
                    # last image, first half: store right here (slack in the
                    # cast chain); the final half's stores are deferred below
                    # so the last casts aren't delayed by store issue
                    if (img == LAST and cb == 1 and t % P2 == P2 - 1):
                        emit_store(scalar, "scalar", img, zc, 0)
                        n_store += 1
                # drain: last-half stores of the last image, per column
                # block, after all casts (waits are already satisfied)
                for cb in (2, 3):
                    for zc in range(NCHUNK):
                        if CAST_ENG[zc] != "a":
                            continue
                        scalar.dma_start(
                            out[zc].ap()[LAST, :, cb * CB:(cb + 1) * CB],
                            o_sb[LAST][zc][:, cb * CB:(cb + 1) * CB],
                        ).then_inc(s_st["scalar"], 16)
                        n_store += 1
                scalar.wait_ge(s_st["scalar"], 16 * n_store)

            @blk.tensor
            def _(tensor):
                # bridge the warmup across the block barrier: keep the PE
                # busy until the first loads' semaphores land, else the
                # ~2.5us idle gap re-arms the cold throttle
                for _ in range(22):
                    tensor.matmul(psum[7][:, :P], a_sb[0][:, :P],
                                  a_sb[0][:, :P], start=True, stop=True)
                tensor.wait_ge(s_gt, 16)
                for img in range(IMGS):
                    for cb in range(NCB):
                        tensor.wait_ge(s_ld[img % 2][cb],
                                       16 * (img // 2 + 1))
                        for zc in range(NCHUNK):
                            for p2 in range(P2):
                                t = tidx(img, cb, zc, p2)
                                if t >= 8:
                                    e = CAST_ENG[zc]
                                    tensor.wait_ge(s_cast[e],
                                                   _cnt[e][t - 8])
                                p = cb * P2 + p2
                                for i, d in enumerate(KEPT_D):
                                    jc = (zc - d) % NCHUNK
                                    mm = tensor.matmul(
                                        psum[t % 8][:, :PT],
                                        gt_sb[:, (zc * NKEPT + i) * P:
                                              (zc * NKEPT + i + 1) * P],
                                        a_sb_v[img % 2][
                                            :, jc, p * PT:(p + 1) * PT],
                                        start=(i == 0), stop=(i == NKEPT - 1),
                                    )
                                mm.then_inc(s_mm)

            @blk.vector
            def _(vector):
                for t in range(NTILES):
                    zc = zc_of(t)
                    if CAST_ENG[zc] != "v":
                        continue
                    vector.wait_ge(s_mm, t + 1)
                    img, cb = t // TILES_PER_IMG, \
                        (t % TILES_PER_IMG) // TILES_PER_CB
                    p = cb * P2 + (t % P2)
                    vector.tensor_copy(
                        o_sb[img][zc][:, p * PT:(p + 1) * PT],
                        psum[t % 8][:, :PT],
                    ).then_inc(s_cast["v"])

    nc.compile()
    return nc


def _make_gt(inhib_kernel: np.ndarray) -> np.ndarray:
    """Packed stationary blocks: col block (zc*NKEPT+i) holds
    GTs[jc*P:(jc+1)*P, zc*P:(zc+1)*P] with jc=(zc-KEPT_D[i])%NCHUNK,
    where GTs[j, r] = g[(r + ROT - j) mod C]."""
    k = np.asarray(inhib_kernel, dtype=np.float64)
    g = np.real(np.fft.ifft(1.0 / np.fft.fft(k)))
    gts = g[(np.arange(C)[None, :] + ROT - np.arange(C)[:, None]) % C]
    gtp = np.empty((P, NCHUNK * NKEPT * P), dtype=NP_OF[W_DT])
    for zc in range(NCHUNK):
        for i, d in enumerate(KEPT_D):
            jc = (zc - d) % NCHUNK
            b = zc * NKEPT + i
            gtp[:, b * P:(b + 1) * P] = gts[jc * P:(jc + 1) * P,
                                            zc * P:(zc + 1) * P]
    return np.ascontiguousarray(gtp)


def make_in_maps(activations, inhib_kernel):
    acts = np.asarray(activations, dtype=np.float32).reshape(N, C, HW)
    acts8 = acts.astype(NP_OF[A_DT])
    gtp = _make_gt(np.asarray(inhib_kernel))
    return [
        {"act": np.ascontiguousarray(acts8[c * IMGS:(c + 1) * IMGS]),
         "gtp": gtp}
        for c in range(N_CORES)
    ]


def kernel(activations, inhib_kernel):
    acts = np.asarray(activations, dtype=np.float32)
    assert acts.shape == (N, C, H, W), acts.shape

    if "nc" not in _CACHE:
        _CACHE["nc"] = _build_nc()
    nc = _CACHE["nc"]

    in_maps = make_in_maps(acts, inhib_kernel)
    res = run_bass_kernel_spmd(nc, in_maps, core_ids=list(range(N_CORES)))
    z = np.concatenate(
        [np.concatenate([r[f"out{zc}"].astype(np.float32)
                         for zc in range(NCHUNK)], axis=1)
         for r in res.results], axis=0)
    # un-rotate: y[i] = z[(i - ROT) mod C]
    y = z[:, (np.arange(C) - ROT) % C, :]
    return y.reshape(N, C, H, W)


# revision 22
# speedup vs baseline: 1.2347x; 1.0390x over previous
"""ConvergedInhibition TRN2 kernel.

The reference computes, per pixel (n,h,w), an FFT deconvolution along the
channel axis: y = ifft(fft(x)/fft(k)).real. Since k is fixed, this is a
circular convolution with g = ifft(1/fft(k)): y[i] = sum_j g[(i-j) mod C] x[j]
— a dense CxC circulant matmul applied to every pixel. Viewing activations[n]
as a [C, H*W] matrix A_n, the problem is out_n = G @ A_n: a [512,512] x
[512,3136] matmul per image, data-parallel over 32 images across 8 cores.

Implementation choices (measured on HW):
- The deconv kernel g is concentrated in a ~224-wide circular window.
  Rotating output rows by S=288 (z[r] = y[(r+S) mod C]) aligns the support
  so only 2 of 4 K-chunks of the contraction carry mass (each output row
  keeps a 256-wide sliding window of g; truncation costs ~2e-3 rel).
  The rotation is undone by a host-side gather.
- fp8 (e3m4) I/O: |x| < 6 << 15.5 = e3m4 max, 4 mantissa bits -> ~1.34e-2
  rms rounding per side (measured end-to-end rel err 1.907e-2, HW matches
  the numpy simulation exactly). Per-zc output dtype stays configurable.
  Weights are fp16 (PE upcasts operands to FP22, mixed dtypes allowed).
- Only the 8 needed [128,128] weight blocks ship (256 KB, one DMA).
- Each dma_start occupies its HWDGE ring ~630ns regardless of size, so
  DMAs are as large as possible: one per (img, cb) on loads (400KB, 3D
  access pattern covering all 4 channel chunks), half-image-width stores.
  All loads+stores ride the sync ring; gt + the first block are prefetched
  from the semaphore-clears block so their sems fire before the main
  block's barrier lifts.
- PSUM->out casts are split between DVE (zc 0,1) and ACT (zc 2,3), each
  with its own completion semaphore (bank t%8 is always cast by the same
  engine since t and t-8 share (zc, p2)), so cast rate keeps up with PE.
"""

import numpy as np
import ml_dtypes

import concourse.bass as bass  # noqa: F401  (registers bass types)
import concourse.mybir as mybir
from concourse import bacc
from concourse.bass_utils import run_bass_kernel_spmd

N_CORES = 8
N, C, H, W = 32, 512, 56, 56
HW = H * W                      # 3136
IMGS = N // N_CORES             # 4 images per core
P = 128                         # partitions
NCHUNK = C // P                 # 4
PT = 392                        # pixel tile (free dim), 3136 = 8*392
NPT = HW // PT                  # 8
CB = 784                        # column block, 3136 = 4*784
NCB = HW // CB                  # 4
ROT = 288                       # output-row rotation aligning g's support
KEPT_D = (0, 1)                 # kept (zc - jc) mod 4 chunk distances
NKEPT = len(KEPT_D)

F8 = mybir.dt.float8e3
F16 = mybir.dt.float16
NP_OF = {F8: ml_dtypes.float8_e3m4, F16: np.float16}
A_DT = F8                       # activation (input) dtype
OUT_DT = [F8, F8, F8, F8]       # output dtype per output chunk zc
W_DT = F16                      # gt weights
CAST_ENG = ["v", "v", "a", "a"]  # cast engine per zc (v=DVE, a=ACT)

P2 = NPT // NCB                       # p-tiles per column block (2)
TILES_PER_CB = NCHUNK * P2            # 8 psum tiles per (img, cb)
TILES_PER_IMG = NCB * TILES_PER_CB    # 32
NTILES = IMGS * TILES_PER_IMG         # 128


def tidx(img, cb, zc, p2):
    return img * TILES_PER_IMG + cb * TILES_PER_CB + zc * P2 + p2


def zc_of(t):
    return (t % TILES_PER_CB) // P2


# cnt_eng[e][t]: number of tiles t' <= t whose cast runs on engine e
_cnt = {"v": [0] * NTILES, "a": [0] * NTILES}
_c = {"v": 0, "a": 0}
for _t in range(NTILES):
    _c[CAST_ENG[zc_of(_t)]] += 1
    _cnt["v"][_t] = _c["v"]
    _cnt["a"][_t] = _c["a"]

_CACHE = {}


def _build_nc():
    nc = bacc.Bacc("TRN2", target_bir_lowering=False, debug=False,
                   num_devices=N_CORES)
    # host pre-arranges activations partition-major and cb-major
    # ([img, p, cb, jc, m] flattened) so every (img, cb) load is a single
    # 2D DMA with 3136B-contiguous runs on both the HBM and SBUF side
    act = nc.dram_tensor("act", [IMGS, P, NCHUNK * HW], A_DT,
                         kind="ExternalInput")
    gtp = nc.dram_tensor("gtp", [P, NCHUNK * NKEPT * P], W_DT,
                         kind="ExternalInput")
    out = [nc.dram_tensor(f"out{zc}", [IMGS, P, HW], OUT_DT[zc],
                          kind="ExternalOutput") for zc in range(NCHUNK)]

    CBW = NCHUNK * CB  # 3136 cols per (cb) block in the cb-major layout

    from contextlib import ExitStack
    with ExitStack() as ctx:
        a_sb = [ctx.enter_context(
            nc.sbuf_tensor(f"a_sb{h}", [P, NCHUNK * HW], A_DT)).ap()
            for h in range(2)]
        gt_sb = ctx.enter_context(
            nc.sbuf_tensor("gt_sb", [P, NCHUNK * NKEPT * P], W_DT)).ap()
        o_sb = [[ctx.enter_context(
            nc.sbuf_tensor(f"o_sb{i}_{z}", [P, HW], OUT_DT[z])).ap()
            for z in range(NCHUNK)] for i in range(IMGS)]
        psum = [ctx.enter_context(
            nc.psum_tensor(f"ps{i}", [P, 512], mybir.dt.float32)).ap()
            for i in range(8)]

        s_gt = nc.alloc_semaphore("s_gt")
        s_ld = [[nc.alloc_semaphore(f"s_ld{h}_{cb}") for cb in range(NCB)]
                for h in range(2)]
        s_mm = nc.alloc_semaphore("s_mm")
        s_cast = {"v": nc.alloc_semaphore("s_cast_v"),
                  "a": nc.alloc_semaphore("s_cast_a")}
        s_st = {"sync": nc.alloc_semaphore("s_st_sync"),
                "scalar": nc.alloc_semaphore("s_st_sca")}
        all_sems = ([s_gt, s_mm, s_cast["v"], s_cast["a"],
                     s_st["sync"], s_st["scalar"]]
                    + [s for row in s_ld for s in row])

        def emit_load(sync, img, cb):
            # one contiguous-2D DMA: [128 part, 3136 cols]
            if img >= 2:
                sync.wait_ge(s_mm, TILES_PER_IMG * (img - 2)
                             + TILES_PER_CB * (cb + 1))
            sync.dma_start(
                a_sb[img % 2][:, cb * CBW:(cb + 1) * CBW],
                act.ap()[img, :, cb * CBW:(cb + 1) * CBW],
            ).then_inc(s_ld[img % 2][cb], 16)

        def emit_store(eng, ring, img, zc, h2):
            # half-image-width store: cbs {2*h2, 2*h2+1}
            e = CAST_ENG[zc]
            eng.wait_ge(s_cast[e],
                        _cnt[e][tidx(img, 2 * h2 + 1, zc, P2 - 1)])
            eng.dma_start(
                out[zc].ap()[img, :, h2 * 2 * CB:(h2 + 1) * 2 * CB],
                o_sb[img][zc][:, h2 * 2 * CB:(h2 + 1) * 2 * CB],
            ).then_inc(s_st[ring], 16)

        # Stage 0: clear semaphores, then prefetch gt + (img0, cb0) on the
        # sync queue (ordered after the clears on that queue; their sem
        # increments land while the main block is still in its barrier).
        # Sems are NOT zeroed on alloc and must not carry values across
        # executions, hence the clears.
        with nc.Block("clears") as blk:

            @blk.sync
            def _(sync):
                for s in all_sems:
                    sync.sem_clear(s)
                emit_load(sync, 0, 0)

            @blk.scalar
            def _(scalar):
                # gt rides the otherwise-idle scalar ring so its receipt
                # doesn't queue behind the activation packets
                scalar.dma_start(gt_sb[:], gtp.ap()[:]).then_inc(s_gt, 16)

            @blk.tensor
            def _(tensor):
                # HAM warmup on junk SBUF while the clears + first loads
                # issue: the PE's clock gate needs ~3.4us of sustained
                # activity to lift the 1.2GHz cold throttle, so burn that
                # window here where the tensor engine would idle anyway
                # (start=True resets bank 7 before its first real use).
                for _ in range(28):
                    tensor.matmul(psum[7][:, :P], a_sb[0][:, :P],
                                  a_sb[0][:, :P], start=True, stop=True)

        LAST = IMGS - 1  # last image: finer stores, split across both rings

        with nc.Block("main") as blk:

            @blk.sync
            def _(sync):
                for cb in range(1, NCB):
                    emit_load(sync, 0, cb)
                for cb in range(NCB):
                    emit_load(sync, 1, cb)
                n_store = 0
                for img in range(IMGS):
                    for h2 in range(NCB // 2):
                        if img + 2 < IMGS:
                            emit_load(sync, img + 2, 2 * h2)
                            emit_load(sync, img + 2, 2 * h2 + 1)
                        for zc in range(NCHUNK):
                            if img == LAST and CAST_ENG[zc] == "a":
                                continue  # on the scalar ring
                            if img == LAST and h2 == 1:
                                continue  # finer drain stores below
                            emit_store(sync, "sync", img, zc, h2)
                            n_store += 1
                # drain: per-column-block stores for the last image's final
                # half so the closing transfer+receipt is as short as
                # possible; sync also takes zc2 (an ACT-cast chunk) so the
                # scalar queue only drains zc3 behind its final casts
                for cb in (2, 3):
                    for zc in (0, 1, 2):
                        e = CAST_ENG[zc]
                        sync.wait_ge(s_cast[e],
                                     _cnt[e][tidx(LAST, cb, zc, P2 - 1)])
                        sync.dma_start(
                            out[zc].ap()[LAST, :, cb * CB:(cb + 1) * CB],
                            o_sb[LAST][zc][:, cb * CB:(cb + 1) * CB],
                        ).then_inc(s_st["sync"], 16)
                        n_store += 1
                sync.wait_ge(s_st["sync"], 16 * n_store)

            @blk.scalar
            def _(scalar):
                n_store = 0
                for t in range(NTILES):
                    zc = zc_of(t)
                    if CAST_ENG[zc] != "a":
                        continue
                    scalar.wait_ge(s_mm, t + 1)
                    img, cb = t // TILES_PER_IMG, \
                        (t % TILES_PER_IMG) // TILES_PER_CB
                    p = cb * P2 + (t % P2)
                    scalar.copy(
                        o_sb[img][zc][:, p * PT:(p + 1) * PT],
                        psum[t % 8][:, :PT],
                    ).then_inc(s_cast["a"])
                    # last image, first half: store right here (slack in the
                    # cast chain); the final half's stores are deferred below
                    # so the last casts aren't delayed by store issue
                    if (img == LAST and cb == 1 and t % P2 == P2 - 1):
                        emit_store(scalar, "scalar", img, zc, 0)
                        n_store += 1
                # drain: last-half zc3 stores, per column block, after all
                # casts (waits already satisfied; zc2's went to sync)
                for cb in (2, 3):
                    zc = 3
                    scalar.dma_start(
                        out[zc].ap()[LAST, :, cb * CB:(cb + 1) * CB],
                        o_sb[LAST][zc][:, cb * CB:(cb + 1) * CB],
                    ).then_inc(s_st["scalar"], 16)
                    n_store += 1
                scalar.wait_ge(s_st["scalar"], 16 * n_store)

            @blk.tensor
            def _(tensor):
                # bridge the warmup across the block barrier: keep the PE
                # busy until the first loads' semaphores land, else the
                # ~2.5us idle gap re-arms the cold throttle
                for _ in range(22):
                    tensor.matmul(psum[7][:, :P], a_sb[0][:, :P],
                                  a_sb[0][:, :P], start=True, stop=True)
                tensor.wait_ge(s_gt, 16)
                for img in range(IMGS):
                    for cb in range(NCB):
                        tensor.wait_ge(s_ld[img % 2][cb],
                                       16 * (img // 2 + 1))
                        for zc in range(NCHUNK):
                            for p2 in range(P2):
                                t = tidx(img, cb, zc, p2)
                                if t >= 8:
                                    e = CAST_ENG[zc]
                                    tensor.wait_ge(s_cast[e],
                                                   _cnt[e][t - 8])
                                for i, d in enumerate(KEPT_D):
                                    jc = (zc - d) % NCHUNK
                                    c0 = cb * CBW + jc * CB + p2 * PT
                                    mm = tensor.matmul(
                                        psum[t % 8][:, :PT],
                                        gt_sb[:, (zc * NKEPT + i) * P:
                                              (zc * NKEPT + i + 1) * P],
                                        a_sb[img % 2][:, c0:c0 + PT],
                                        start=(i == 0), stop=(i == NKEPT - 1),
                                    )
                                mm.then_inc(s_mm)

            @blk.vector
            def _(vector):
                for t in range(NTILES):
                    zc = zc_of(t)
                    if CAST_ENG[zc] != "v":
                        continue
                    vector.wait_ge(s_mm, t + 1)
                    img, cb = t // TILES_PER_IMG, \
                        (t % TILES_PER_IMG) // TILES_PER_CB
                    p = cb * P2 + (t % P2)
                    vector.tensor_copy(
                        o_sb[img][zc][:, p * PT:(p + 1) * PT],
                        psum[t % 8][:, :PT],
                    ).then_inc(s_cast["v"])

    nc.compile()
    return nc


def _make_gt(inhib_kernel: np.ndarray) -> np.ndarray:
    """Packed stationary blocks: col block (zc*NKEPT+i) holds
    GTs[jc*P:(jc+1)*P, zc*P:(zc+1)*P] with jc=(zc-KEPT_D[i])%NCHUNK,
    where GTs[j, r] = g[(r + ROT - j) mod C]."""
    k = np.asarray(inhib_kernel, dtype=np.float64)
    g = np.real(np.fft.ifft(1.0 / np.fft.fft(k)))
    gts = g[(np.arange(C)[None, :] + ROT - np.arange(C)[:, None]) % C]
    gtp = np.empty((P, NCHUNK * NKEPT * P), dtype=NP_OF[W_DT])
    for zc in range(NCHUNK):
        for i, d in enumerate(KEPT_D):
            jc = (zc - d) % NCHUNK
            b = zc * NKEPT + i
            gtp[:, b * P:(b + 1) * P] = gts[jc * P:(jc + 1) * P,
                                            zc * P:(zc + 1) * P]
    return np.ascontiguousarray(gtp)


def make_in_maps(activations, inhib_kernel):
    acts = np.asarray(activations, dtype=np.float32).reshape(N, C, HW)
    acts8 = acts.astype(NP_OF[A_DT])
    # [n, (jc p), (cb m)] -> [n, p, (cb jc m)]: partition-major, cb-major
    # so each (img, cb) device load is one fully contiguous 2D transfer
    acts8 = acts8.reshape(N, NCHUNK, P, NCB, CB).transpose(0, 2, 3, 1, 4)
    acts8 = np.ascontiguousarray(acts8).reshape(N, P, NCHUNK * HW)
    gtp = _make_gt(np.asarray(inhib_kernel))
    return [
        {"act": acts8[c * IMGS:(c + 1) * IMGS], "gtp": gtp}
        for c in range(N_CORES)
    ]


def kernel(activations, inhib_kernel):
    acts = np.asarray(activations, dtype=np.float32)
    assert acts.shape == (N, C, H, W), acts.shape

    if "nc" not in _CACHE:
        _CACHE["nc"] = _build_nc()
    nc = _CACHE["nc"]

    in_maps = make_in_maps(acts, inhib_kernel)
    res = run_bass_kernel_spmd(nc, in_maps, core_ids=list(range(N_CORES)))
    z = np.concatenate(
        [np.concatenate([r[f"out{zc}"].astype(np.float32)
                         for zc in range(NCHUNK)], axis=1)
         for r in res.results], axis=0)
    # un-rotate: y[i] = z[(i - ROT) mod C]
    y = z[:, (np.arange(C) - ROT) % C, :]
    return y.reshape(N, C, H, W)
